# revision 1
# baseline (speedup 1.0000x reference)
"""Bahdanau additive attention kernel for Trainium2 (8 NeuronCores).

Reference computation (B=32, S=4096, D=512):
    pre   = enc @ We.T + (hidden @ Wh.T + b1)[:, None, :]   # [B, S, D]
    h     = tanh(pre)
    e     = h @ w2                                          # [B, S]
    alpha = softmax(e, axis=1)
    ctx   = einsum('bs,bsd->bd', alpha, enc)                # [B, D]

Strategy: data-parallel over batch (4 batches per core). Inputs are
re-laid-out on host so the device only ever does efficient, contiguous
DMA:
  - enc is passed transposed per-batch as [st, di, p, s] bf16 so the
    contraction dim d sits on SBUF partitions for the PE matmuls.
  - The big matmul runs in bf16 (1 cycle/row on PE vs 4 for fp32);
    accumulation is fp32 in PSUM. Verified rel-l2 error ~2.6e-3.
  - e rows are accumulated in PSUM via M=1 matmuls with w2 stationary,
    then DMA'd into an [8, 512] SBUF tile per batch for the softmax.
  - context^T is accumulated on the DVE with fused multiply+reduce
    (tensor_tensor_reduce) against partition-broadcast alpha rows.
"""

import sys

if "/opt/trn_rl_repo" not in sys.path:
    sys.path.insert(0, "/opt/trn_rl_repo")

from contextlib import ExitStack

import ml_dtypes
import numpy as np

import concourse.bass as bass
import concourse.bacc as bacc
import concourse.tile as tile
from concourse import mybir
from concourse.bass_utils import run_bass_kernel_spmd

B, S, D = 32, 4096, 512
NCORES = 8
BPC = B // NCORES          # batches per core
P = 128                    # partitions
NDC = D // P               # d (contraction) chunks
NKC = D // P               # k (output channel) chunks
ST = 1024                  # s-tile size (PE/ACT/DVE granularity)
NST = S // ST              # s tiles per batch
EST = 512                  # e-row granularity (one PSUM bank)
NER = S // EST             # exp rows per batch

F32 = mybir.dt.float32
BF16 = mybir.dt.bfloat16
AF = mybir.ActivationFunctionType
ALU = mybir.AluOpType


def build_bass():
    nc = bacc.Bacc()

    encT = nc.declare_dram_parameter("encT", [BPC, NST, NDC, P, ST], BF16, isOutput=False)
    weT = nc.declare_dram_parameter("weT", [NDC, P, D], BF16, isOutput=False)
    whT = nc.declare_dram_parameter("whT", [NDC, P, D], F32, isOutput=False)
    hT = nc.declare_dram_parameter("hT", [NDC, P, BPC], F32, isOutput=False)
    b1r = nc.declare_dram_parameter("b1r", [P, NKC], F32, isOutput=False)
    w2r = nc.declare_dram_parameter("w2r", [P, NKC], BF16, isOutput=False)
    ctx_out = nc.declare_dram_parameter("ctx", [P, NDC, BPC], F32, isOutput=True)

    with TileKernel(nc) as tk:
        tk.build(encT, weT, whT, hT, b1r, w2r, ctx_out)
    nc.finalize()
    return nc


class TileKernel:
    def __init__(self, nc):
        self.nc = nc
        self.stack = ExitStack()
        self.tc = None

    def __enter__(self):
        self.tc = self.stack.enter_context(tile.TileContext(self.nc))
        return self

    def __exit__(self, *exc):
        return self.stack.__exit__(*exc)

    def build(self, encT, weT, whT, hT, b1r, w2r, ctx_out):
        nc, tc, ctx = self.nc, self.tc, self.stack

        singles = ctx.enter_context(tc.tile_pool(name="singles", bufs=1))
        encp = ctx.enter_context(tc.tile_pool(name="encp", bufs=2 * NST))
        htp = ctx.enter_context(tc.tile_pool(name="htp", bufs=4))
        abp = ctx.enter_context(tc.tile_pool(name="abp", bufs=8))
        junkp = ctx.enter_context(tc.tile_pool(name="junkp", bufs=3))
        smp = ctx.enter_context(tc.tile_pool(name="smp", bufs=3))
        cpool = ctx.enter_context(tc.tile_pool(name="cpool", bufs=2))
        ctxp = ctx.enter_context(tc.tile_pool(name="ctxp", bufs=2))
        dramp = ctx.enter_context(tc.tile_pool(name="dramp", bufs=2, space="DRAM"))
        prep = ctx.enter_context(tc.tile_pool(name="prep", bufs=3, space="PSUM"))
        ecp = ctx.enter_context(tc.tile_pool(name="ecp", bufs=2, space="PSUM"))

        # ---- load constants ----
        w_sb = singles.tile([P, NDC, D], BF16)
        nc.sync.dma_start(out=w_sb, in_=weT[:].rearrange("di p k -> p di k"))
        wh_sb = singles.tile([P, NDC, D], F32)
        nc.sync.dma_start(out=wh_sb, in_=whT[:].rearrange("di p k -> p di k"))
        h_sb = singles.tile([P, NDC, BPC], F32)
        nc.sync.dma_start(out=h_sb, in_=hT[:].rearrange("di p b -> p di b"))
        b1_sb = singles.tile([P, NKC], F32)
        nc.sync.dma_start(out=b1_sb, in_=b1r[:])
        w2_sb = singles.tile([P, NKC], BF16)
        nc.sync.dma_start(out=w2_sb, in_=w2r[:])

        # ---- PE warm-up burst ----
        # ~7 us of dummy matmuls while the first enc tiles stream in, so the
        # HAM clock gate reaches 8/8 before real work starts.
        wpre = prep.tile([P, ST], F32, tag="pre")
        for i in range(24):
            nc.tensor.matmul(
                out=wpre[:, 0:D], lhsT=w_sb[:, 0, 0:P], rhs=w_sb[:, i % NDC, :],
                start=True, stop=True,
            )
        wjunk = singles.tile([P, 1], F32)
        nc.vector.tensor_copy(out=wjunk, in_=wpre[:, 0:1])

        # ---- c^T = Wh @ hidden^T + b1 on PE, laid out [k(part), ki, b] ----
        # Tiny fp32 matmuls (N=4) accumulated over the 4 d-chunks; runs
        # during the warm-up/DMA phase. Bacc's sync-wait legalization makes
        # fp32 matmuls safe here.
        cps = ecp.tile([P, NKC * BPC], F32, tag="ec")
        for ki in range(NKC):
            for di in range(NDC):
                nc.tensor.matmul(
                    out=cps[:, ki * BPC:(ki + 1) * BPC],
                    lhsT=wh_sb[:, di, ki * P:(ki + 1) * P],
                    rhs=h_sb[:, di, :],
                    start=(di == 0),
                    stop=(di == NDC - 1),
                )
        c_sb = singles.tile([P, NKC, BPC], F32)
        for ki in range(NKC):
            nc.vector.tensor_scalar_add(
                out=c_sb[:, ki, :],
                in0=cps[:, ki * BPC:(ki + 1) * BPC],
                scalar1=b1_sb[:, ki:ki + 1],
            )

        # ---- main per-batch pipeline ----
        # Softmax runs UNNORMALIZED and streamed: e is bounded (|e| < ~5)
        # so exp needs no max pass; exp(e) for each 512-wide e row is
        # computed as soon as it lands in PSUM, round-trips through DRAM
        # to partition-broadcast, and feeds the context accumulation while
        # the PE is still working on later s-tiles. The 1/sum(exp)
        # normalization is applied once to the final [128, NDC] context.
        for b in range(BPC):
            pd = dramp.tile([NER, EST], BF16, tag="pd")
            lparts = smp.tile([1, NER], F32, tag="lparts")
            cacc = ctxp.tile([P, NDC, NST], F32, tag="cacc")
            for st in range(NST):
                et = encp.tile([P, NDC, ST], BF16, tag="et")
                nc.sync.dma_start(out=et, in_=encT[:][b, st].rearrange("di p s -> p di s"))

                ht = htp.tile([P, NKC, ST], BF16, tag="ht")
                for ki in range(NKC):
                    pre_ps = prep.tile([P, ST], F32, tag="pre")
                    for half in range(ST // EST):
                        sl = slice(half * EST, (half + 1) * EST)
                        for di in range(NDC):
                            nc.tensor.matmul(
                                out=pre_ps[:, sl],
                                lhsT=w_sb[:, di, ki * P:(ki + 1) * P],
                                rhs=et[:, di, sl],
                                start=(di == 0),
                                stop=(di == NDC - 1),
                            )
                    # h^T = tanh(pre^T + c), one [128, ST] ACT op per ki
                    nc.scalar.activation(
                        out=ht[:, ki, :],
                        in_=pre_ps,
                        func=AF.Tanh,
                        bias=c_sb[:, ki, b:b + 1],
                        scale=1.0,
                    )
                for half in range(ST // EST):
                    sl = slice(half * EST, (half + 1) * EST)
                    r = st * (ST // EST) + half
                    e_ps = ecp.tile([1, EST], F32, tag="ec")
                    for ki in range(NKC):
                        nc.tensor.matmul(
                            out=e_ps,
                            lhsT=w2_sb[:, ki:ki + 1],
                            rhs=ht[:, ki, sl],
                            start=(ki == 0),
                            stop=(ki == NKC - 1),
                        )
                    # p = exp(e) with the row-sum fused; bf16 row goes out
                    # through DRAM so DMA can replicate it across partitions.
                    p_row = smp.tile([1, EST], BF16, tag="prow")
                    nc.scalar.activation(
                        out=p_row, in_=e_ps, func=AF.Exp, bias=0.0, scale=1.0,
                        accum_out=lparts[:, r:r + 1],
                    )
                    nc.gpsimd.dma_start(out=pd[r:r + 1, :], in_=p_row)
                # broadcast the two p rows of this s-tile in one DMA (rows
                # are contiguous in DRAM) and accumulate p * enc on DVE.
                ab = abp.tile([P, ST], BF16, tag="ab")
                rows = pd[st * (ST // EST):(st + 1) * (ST // EST), :]
                nc.gpsimd.dma_start(
                    out=ab,
                    in_=bass.AP(
                        tensor=rows.tensor,
                        offset=rows.offset,
                        ap=[[0, P], [1, ST]],
                    ),
                )
                for di in range(NDC):
                    junk = junkp.tile([P, ST], BF16, tag="junk")
                    nc.vector.scalar_tensor_tensor(
                        out=junk,
                        in0=et[:, di, :],
                        scalar=1.0,
                        in1=ab,
                        op0=ALU.mult,
                        op1=ALU.mult,
                        accum_out=cacc[:, di, st:st + 1],
                    )

            # ---- finalize: ctx = (sum_s p*enc) / sum_s p ----
            lsum = smp.tile([1, 1], F32, tag="lsum")
            nc.vector.reduce_sum(out=lsum, in_=lparts, axis=mybir.AxisListType.X)
            rinv1 = smp.tile([1, 1], F32, tag="rinv1")
            nc.vector.reciprocal(out=rinv1, in_=lsum)
            rinvb = smp.tile([P, 1], F32, tag="rinvb")
            nc.gpsimd.partition_broadcast(out_ap=rinvb, in_ap=rinv1)
            ctx_acc = ctxp.tile([P, NDC], F32, tag="ctx")
            nc.vector.reduce_sum(out=ctx_acc, in_=cacc, axis=mybir.AxisListType.X)
            nc.vector.tensor_scalar_mul(out=ctx_acc, in0=ctx_acc, scalar1=rinvb)
            nc.gpsimd.dma_start(out=ctx_out[:][:, :, b], in_=ctx_acc)


_NC_CACHE = None


def _get_nc():
    global _NC_CACHE
    if _NC_CACHE is None:
        _NC_CACHE = build_bass()
    return _NC_CACHE


def _prep_core_inputs(hidden_state, encoder_outputs, W1, b1, w2, core):
    bf16 = ml_dtypes.bfloat16
    b0 = core * BPC
    enc = encoder_outputs[b0:b0 + BPC]                      # [BPC, S, D] f32
    # [b, d, s] -> [b, di, p, s] -> [b, st, di, p, s]
    e = enc.transpose(0, 2, 1).reshape(BPC, NDC, P, NST, ST)
    e = np.ascontiguousarray(e.transpose(0, 3, 1, 2, 4)).astype(bf16)
    return {
        "encT": e,
        "weT": np.ascontiguousarray(W1[:, :D].T.reshape(NDC, P, D)).astype(bf16),
        "whT": np.ascontiguousarray(W1[:, D:].T.reshape(NDC, P, D)),
        "hT": np.ascontiguousarray(hidden_state[b0:b0 + BPC].T.reshape(NDC, P, BPC)),
        "b1r": np.ascontiguousarray(b1.reshape(NKC, P).T),
        "w2r": np.ascontiguousarray(w2.reshape(NKC, P).T).astype(bf16),
    }


def kernel(hidden_state, encoder_outputs, W1, b1, w2, _trace=False, _trace_kwargs=None):
    hidden_state = np.asarray(hidden_state, dtype=np.float32)
    encoder_outputs = np.asarray(encoder_outputs, dtype=np.float32)
    W1 = np.asarray(W1, dtype=np.float32)
    b1 = np.asarray(b1, dtype=np.float32)
    w2 = np.asarray(w2, dtype=np.float32)

    nc = _get_nc()
    in_maps = [
        _prep_core_inputs(hidden_state, encoder_outputs, W1, b1, w2, c)
        for c in range(NCORES)
    ]
    res = run_bass_kernel_spmd(
        nc, in_maps, list(range(NCORES)), trace=_trace,
        **(_trace_kwargs or {}),
    )
    out = np.empty((B, D), dtype=np.float32)
    for c in range(NCORES):
        r = res.results[c]["ctx"]                          # [p, di, b]
        out[c * BPC:(c + 1) * BPC] = r.transpose(2, 1, 0).reshape(BPC, D)
    if _trace:
        return out, res
    return out

